# revision 19
# baseline (speedup 1.0000x reference)
# Multi-head attention + output projection kernel for 8 TRN2 NeuronCores.
#
# Problem: q,k,v [4,16,2048,64] fp32; w_out [64,64]; b_out [64]
#   out = softmax(q @ k^T / sqrt(64)) @ v @ w_out^T + b_out
#
# Strategy:
#  - 64 (batch, head) pairs sharded 8-per-core (pure data parallel, no collectives).
#  - Host-side layout prep (fp16): Q^T duplicated on both partition halves, K^T
#    packed so chunk c and c+8 sit in partition rows 0-63 / 64-127 (d=64 on
#    partitions), V with a ones-column appended (softmax denominator rides the
#    PE matmul), w_out^T padded to [128,65] with row 64 = [b_out | 1] and rows
#    65-127 zero, so bias + row-sum ride the projection matmul:
#      proj[q,o] = sum_e y[e,q] w[o,e] + rowsum[q]*b[o];  out = proj / rowsum.
#  - EVERY matmul runs in the same 2x64-row tile_position configuration; the
#    measured ~100-120ns PE penalty on each tiled<->untiled CTRL_PE mode flip
#    (2 per 2-unit round in the v1 kernel, ~28us total) disappears:
#      * scores: chunk pair (c, c+8) on PE row halves, contract d=64 each.
#      * y: each chunk's kv-contraction (128) split into kv 0-63 (rows 0-63,
#        accumulating into yA) and kv 64-127 (rows 64-127 -> yB). The halves
#        stream the same attn tile concurrently; yA+yB are merged by the
#        PSUM->SBUF cast, which becomes a VectorE tensor_add.
#      * projection: per q-tile an accumulating pair: rows 0-63 contract the
#        64 value dims, rows 64-127 contract the den row (+ zero padding).
#  - exp alternates whole [128,1024] scores tiles between ScalarE (exact
#    table exp) and VectorE (one-instruction Schraudolph approximation in fp16
#    bit space), ~13:19 per 32 units: the DVE also carries the yA+yB merge and
#    the normalize, ScalarE only exp. No max-subtraction: |scores/8| < ~7.
#  - Startup: kt chunk 0 rides the (idle until first MM) Tensor DMA queue so
#    its descriptor generation overlaps qt block 0's on Sync. The final
#    block's output DMA also uses the Tensor queue (PE is done by then),
#    shortening the drain tail behind Sync's teardown work.

import math

import numpy as np

import concourse.mybir as mybir
import concourse.tile as tile
from concourse import bacc
from concourse.bass_utils import run_bass_kernel_spmd

F16 = mybir.dt.float16
F32 = mybir.dt.float32

B, H, S, D = 4, 16, 2048, 64
N_CORES = 8
N_HEADS = B * H                    # 64
HPC = N_HEADS // N_CORES           # 8 heads per core
SCALE = 1.0 / math.sqrt(D)         # 1/8

# fp16 Schraudolph exp for the VectorE path: exp(s/8) ~= bitcast_f16(
# int16_rne(s * EXPA + EXPB)) (HW converts round-to-nearest). Max rel err
# ~3.0%; softmax normalization and 2048-term averaging wash it to ~1e-2
# end-to-end on the DVE-covered stripes.
EXPA = 1024.0 / (8.0 * math.log(2.0))
EXPB = 15315.5

# Units (mod 8, i.e. block-relative) whose exp runs on VectorE via
# Schraudolph; the rest use ScalarE table exp. Whole-unit assignment
# keeps each downstream matmul waiting on a single engine. 3-of-8 =
# 96 DVE / 160 ACT units balances the engines (~170us each) given the
# DVE also carries the merge + normalize. The slot CHOICE is phase
# tuned: the block-m merge lands on the DVE around block-relative
# unit 4 of block m+1, so slots {1,4,6} keep the queue shallow there
# while still meeting each exp's sc-ring deadline (exp(u) must retire
# before scores(u+3)).
DVE_PAT = frozenset((1, 4, 6))
# Units of y-job deferral behind scores/exp so the in-order PE stream
# never waits on exp.
DELAY = 4

TRACE = False
TRACE_KWARGS = {}
LAST_RESULT = None

_CACHED = {}


def build_bass(hpc=HPC, seq=S, dve_pat=DVE_PAT):
    """Build the per-core Bass program. Parameterized so a small config can be
    simulated in CoreSim. Requires seq % 256 == 0."""
    QB = min(512, seq)             # q columns per y-accumulation block
    n_m = seq // QB                # y blocks per head
    n_ch = seq // 128              # k chunks per head
    half = n_ch // 2               # chunk pairs per head
    n_qt = QB // 128               # 128-row q tiles per block

    nc = bacc.Bacc("TRN2", target_bir_lowering=False, debug=False)

    qt_d = nc.dram_tensor("qt", [hpc, 128, seq], F16, kind="ExternalInput").ap()
    kt_d = nc.dram_tensor("kt", [hpc, 128, half * 128], F16, kind="ExternalInput").ap()
    vx_d = nc.dram_tensor("vx", [hpc, 128, n_ch, 65], F16, kind="ExternalInput").ap()
    wx_d = nc.dram_tensor("wx", [65, 65], F16, kind="ExternalInput").ap()
    out_d = nc.dram_tensor("out", [hpc, seq, 64], F32, kind="ExternalOutput").ap()

    with tile.TileContext(nc) as tc:
        with (
            tc.tile_pool(name="const", bufs=1) as const_pool,
            tc.tile_pool(name="qk", bufs=3) as qk_pool,
            tc.tile_pool(name="vx", bufs=3) as vx_pool,
            tc.tile_pool(name="attn", bufs=14) as attn_pool,
            tc.tile_pool(name="yext", bufs=4) as yext_pool,
            tc.tile_pool(name="fin", bufs=6) as fin_pool,
            tc.tile_pool(name="psc", bufs=3, space="PSUM") as psum_sc,
            tc.tile_pool(name="psy", bufs=1, space="PSUM") as psum_y,
        ):
            # Head 0's kt chunk 0 rides the Scalar DMA queue, emitted ahead
            # of the exp-table warm-up so its descriptor generation overlaps
            # qt block 0's on Sync (the first scores MM needs both; GpSimd
            # carries the rest of the input stream).
            kt0_sb = qk_pool.tile([128, half * 128], F16, tag="kt")
            nc.scalar.dma_start(kt0_sb[:, 0:128], kt_d[0][:, 0:128])
            # wx's DMA is deferred until after head 0's critical-path
            # loads (it is first read at the first finalize, ~16us in).
            wx_sb = const_pool.tile([65, 65], F16, tag="wx")
            # Pre-warm the ScalarE Exp table: the first real activation
            # otherwise pays the ~1.3us ACT_TABLE_LOAD exactly when the PE
            # is blocked on the 3-deep scores ring at startup.
            warm_sb = const_pool.tile([128, 1], F16, tag="warm")
            nc.vector.memset(warm_sb[:], 0.0)
            nc.scalar.activation(
                warm_sb[:], warm_sb[:],
                mybir.ActivationFunctionType.Exp,
                bias=0.0, scale=SCALE,
            )
            pending = []
            finq = []
            projq = []
            cur_u = [0]
            # Units between the merge emission and the projection MMs: the
            # in-order PE otherwise stalls ~1.9us per block at the proj,
            # waiting for the DVE merge behind the exp backlog (v3 trace:
            # 32 x 1.9us of PE idle at block boundaries).
            PROJ_DELAY = 3

            def merge_stage(ov, yab_ps, u):
                """yA+yB merge-cast on VectorE; proj deferred PROJ_DELAY."""
                y_sb = yext_pool.tile([65, QB], F16, tag="y16")
                # One VectorE pass merges the two kv-half planes of the
                # 2-bank accumulator and casts to the fp16 projection
                # stationary. tensor_reduce keeps it a SINGLE PSUM input
                # (the HW forbids two PSUM operands on one instruction);
                # the 2-element add in fp16 is safe (|y| < ~3e3).
                with nc.allow_low_precision(reason="2-term y merge, |y|<3e3"):
                    nc.vector.tensor_reduce(
                        y_sb[:],
                        yab_ps[:].rearrange("p j q -> p q j"),
                        axis=mybir.AxisListType.X,
                        op=mybir.AluOpType.add,
                    )
                projq.append([ov, y_sb, u])

            def proj_stage(st, last=False):
                """projection -> normalize -> DMA for one block."""
                ov, y_sb, _ = st
                # p borrows a slot in the scores ring (its 1040B fit in a
                # 2-bank sc slot): the yab slot is then gated only on the
                # merge read, never on the (later) normalize. The proj is a
                # single K=65 MM per q-tile (v1 geometry: one CTRL_PE
                # tile-geometry flip in, one out per block, ~30ns MMs).
                p_ps = psum_sc.tile([128, n_qt * 65], F32, tag="sc", name="p_ps")
                for t in range(n_qt):
                    nc.tensor.matmul(
                        p_ps[:, t * 65:(t + 1) * 65],
                        y_sb[:, t * 128:(t + 1) * 128],
                        wx_sb[:],
                        start=True, stop=True,
                    )
                p_v = p_ps[:].rearrange("p (t e) -> p t e", e=65)
                if not last:
                    recip = fin_pool.tile([128, n_qt], F32, tag="recip")
                    nc.vector.reciprocal(recip[:], p_v[:, :, 64])
                    o_sb = fin_pool.tile([128, n_qt, 64], F32, tag="o")
                    nc.vector.tensor_mul(
                        o_sb[:],
                        p_v[:, :, 0:64],
                        recip[:, :, None].to_broadcast((128, n_qt, 64)),
                    )
                    nc.sync.dma_start(ov, o_sb[:])
                    return
                # Last block: normalize + DMA in two halves on the idle
                # Scalar/GpSimd queues so the final data transfer starts
                # ~0.5us earlier and dodges Sync's teardown work.
                hq = n_qt // 2
                for i, q_eng in enumerate((nc.scalar, nc.gpsimd)):
                    ts = slice(i * hq, (i + 1) * hq)
                    recip = fin_pool.tile([128, hq], F32, tag="recip")
                    nc.vector.reciprocal(recip[:], p_v[:, ts, 64])
                    o_sb = fin_pool.tile([128, hq, 64], F32, tag="o")
                    nc.vector.tensor_mul(
                        o_sb[:],
                        p_v[:, ts, 0:64],
                        recip[:, :, None].to_broadcast((128, hq, 64)),
                    )
                    q_eng.dma_start(ov[:, ts], o_sb[:])

            def run_stages(u, last=False):
                if projq and projq[0][2] <= u - PROJ_DELAY:
                    proj_stage(projq.pop(0), last=last)

            def pop_yjob():
                vsb, ov, p, ybox, at_sb = pending.pop(0)
                # The block's 2-bank accumulator is allocated at first use,
                # AFTER the previous block's finalize allocated its p tile:
                # the shared single-slot ring then cycles yab, p, yab', p'
                # with each reuse gated on the merge/normalize reads.
                if not ybox:
                    ybox.append(
                        psum_y.tile([65, 2, QB], F32, tag="yy", name="yab")
                    )
                yab = ybox[0]
                st = p == 0
                sp = p == half - 1
                # kv halves of chunk p stream at_sb[:, 0:QB] concurrently
                # through the two row groups into the two planes (= PSUM
                # banks) of the accumulator.
                nc.tensor.matmul(
                    yab[:, 0, :], vsb[0:64, p], at_sb[0:64, 0:QB],
                    start=st, stop=False,
                )
                nc.tensor.matmul(
                    yab[:, 1, :], vsb[64:128, p], at_sb[64:128, 0:QB],
                    start=st, stop=False,
                )
                nc.tensor.matmul(
                    yab[:, 0, :], vsb[0:64, p + half], at_sb[0:64, QB:2 * QB],
                    start=False, stop=sp,
                )
                nc.tensor.matmul(
                    yab[:, 1, :], vsb[64:128, p + half], at_sb[64:128, QB:2 * QB],
                    start=False, stop=sp,
                )
                if sp:
                    # Merge immediately: every unit between the last yab
                    # write and the merge completing is PE-bubble at the
                    # next block's first y matmuls (single-slot psy ring).
                    merge_stage(ov, yab, cur_u[0])

            u = 0
            for h in range(hpc):
                # Input DMAs ride the (otherwise idle) GpSimd queue so their
                # ~0.7us descriptor-generation cost doesn't serialize behind
                # the output DMAs on Sync. Head 0's kt chunk 0 was issued on
                # the Scalar queue above; qt block 0 goes on Sync.
                if h == 0:
                    kt_sb = kt0_sb
                else:
                    kt_sb = qk_pool.tile([128, half * 128], F16, tag="kt")
                    nc.gpsimd.dma_start(kt_sb[:, 0:128], kt_d[h][:, 0:128])
                qt_sb = qk_pool.tile([128, seq], F16, tag="qt")
                (nc.sync if h == 0 else nc.gpsimd).dma_start(
                    qt_sb[:, 0:QB], qt_d[h][:, 0:QB]
                )
                if h == 0:
                    nc.sync.dma_start(wx_sb[:], wx_d[:])
                nc.gpsimd.dma_start(kt_sb[:, 128:], kt_d[h][:, 128:])
                vx_sb = vx_pool.tile([128, n_ch, 65], F16, tag="vx")
                nc.gpsimd.dma_start(vx_sb[:], vx_d[h])
                for mm_ in range(1, n_m):
                    nc.gpsimd.dma_start(
                        qt_sb[:, mm_ * QB:(mm_ + 1) * QB],
                        qt_d[h][:, mm_ * QB:(mm_ + 1) * QB],
                    )

                # q = m*QB + p*n_qt + t (host-side column permutation):
                # partition p's n_qt rows are consecutive in DRAM.
                out_v = out_d[h].rearrange(
                    "(m p t) o -> m p t o", p=128, t=n_qt
                )
                for m in range(n_m):
                    ybox = []
                    q0 = m * QB
                    for p in range(half):
                        # Pin the Tile scheduler to the emission interleave:
                        # a sim-time floor per unit stops it hoisting ready
                        # y matmuls ahead of semaphore-waiting scores MMs
                        # (the v2 trace showed 20-28-MM y bunches that
                        # starved both exp engines to ~55% occupancy).
                        # Floors only shape the scheduler's simulated order;
                        # the hardware still runs on data semaphores.
                        tc.tile_set_cur_wait(u * 0.0007)
                        kcols = slice(p * 128, (p + 1) * 128)
                        sc_ps = psum_sc.tile([128, 2 * QB], F32, tag="sc")
                        # chunk p on PE rows 0-63, chunk p+half on rows 64-127
                        nc.tensor.matmul(
                            sc_ps[:, 0:QB],
                            kt_sb[0:64, kcols],
                            qt_sb[0:64, q0:q0 + QB],
                            start=True, stop=True,
                        )
                        nc.tensor.matmul(
                            sc_ps[:, QB:2 * QB],
                            kt_sb[64:128, kcols],
                            qt_sb[64:128, q0:q0 + QB],
                            start=True, stop=True,
                        )
                        at_sb = attn_pool.tile([128, 2 * QB], F16, tag="at")
                        if u % 8 in dve_pat:
                            nc.vector.tensor_scalar(
                                at_sb[:].bitcast(mybir.dt.int16),
                                sc_ps[:],
                                EXPA, EXPB,
                                op0=mybir.AluOpType.mult,
                                op1=mybir.AluOpType.add,
                            )
                        else:
                            nc.scalar.activation(
                                at_sb[:], sc_ps[:],
                                mybir.ActivationFunctionType.Exp,
                                bias=0.0, scale=SCALE,
                            )
                        u += 1
                        cur_u[0] = u
                        ov = out_v[m]
                        pending.append((vx_sb, ov, p, ybox, at_sb))
                        run_stages(u)
                        # A block's FIRST unit pops one unit later than the
                        # steady lag: the extra unit is the merge's window
                        # (last yab(m) write + 1.2us DVE merge must finish
                        # before yab(m+1)'s first write hits the shared
                        # PSUM slot). The next unit double-pops to catch up.
                        lim = DELAY + (1 if pending[0][2] == 0 else 0)
                        while len(pending) > lim:
                            pop_yjob()
            # Drain interleaved: finalize blocks as their last y-pops
            # retire so the final merge/proj/normalize/DMA chains overlap
            # the remaining y matmuls instead of serializing after them.
            while pending:
                tc.tile_set_cur_wait(u * 0.0007)
                u += 1
                cur_u[0] = u
                run_stages(u)
                pop_yjob()
            while projq:
                tc.tile_set_cur_wait(u * 0.0007)
                u += 1
                cur_u[0] = u
                run_stages(u, last=(len(projq) == 1))
    nc.finalize()
    return nc


def shard_inputs(q, k, v, w_out, b_out, hpc=HPC, seq=S, n_cores=N_CORES):
    """Host-side layout prep: per-core fp16 transposed shards."""
    n_ch = seq // 128
    half = n_ch // 2
    nh = n_cores * hpc
    qT = np.asarray(q, dtype=np.float32).reshape(nh, seq, D).transpose(0, 2, 1)
    qT = qT.astype(np.float16)                      # [nh, 64, seq]
    # Permute q columns within each 512-block so that projection slice t,
    # lane f carries q = 4f + t: the output tile then holds 4 consecutive
    # q rows per partition, making the out-DMA 1KB-contiguous per lane.
    # Scores/exp/y are elementwise (or reductions) in q, so the shuffle is
    # free; only the out DRAM view must match (see out_v in build_bass).
    qb = min(512, seq)
    n_qt = qb // 128
    # col t*128+f of the shuffled block holds original query f*n_qt + t
    perm = (
        np.arange(128)[None, :] * n_qt + np.arange(n_qt)[:, None]
    ).reshape(qb)
    col_perm = np.concatenate(
        [m * qb + perm for m in range(seq // qb)]
    )
    qT = qT[:, :, col_perm]
    qdup = np.ascontiguousarray(np.concatenate([qT, qT], axis=1))
    kT = np.asarray(k, dtype=np.float32).reshape(nh, seq, D).transpose(0, 2, 1)
    kT = kT.astype(np.float16)                      # [nh, 64, seq]
    kpack = np.ascontiguousarray(np.concatenate(
        [kT[:, :, :half * 128], kT[:, :, half * 128:]], axis=1
    ))

    vf = np.asarray(v, dtype=np.float32).reshape(nh, seq, D)
    vx = np.ones((nh, seq, 65), dtype=np.float16)
    vx[:, :, :64] = vf
    # [nh, 128, n_ch, 65]: partition-major so the SBUF load is one
    # contiguous 2080B row per partition instead of a 130B-element gather.
    vx = np.ascontiguousarray(
        vx.reshape(nh, n_ch, 128, 65).transpose(0, 2, 1, 3)
    )

    # [65, 65]: rows 0-63 = w_out^T, row 64 = [b_out | 1] (den + bias ride
    # the projection's second, K=1 row-group tile).
    wx = np.zeros((65, 65), dtype=np.float16)
    wx[:64, :64] = np.asarray(w_out, dtype=np.float32).T
    wx[64, :64] = np.asarray(b_out, dtype=np.float32)
    wx[64, 64] = 1.0

    in_maps = []
    for c in range(n_cores):
        s0, s1 = c * hpc, (c + 1) * hpc
        in_maps.append({
            "qt": qdup[s0:s1],
            "kt": kpack[s0:s1],
            "vx": vx[s0:s1],
            "wx": wx,
        })
    return in_maps


def kernel(q, k, v, w_out, b_out):
    global LAST_RESULT
    key = "full"
    if key not in _CACHED:
        _CACHED[key] = build_bass()
    nc = _CACHED[key]

    in_maps = shard_inputs(q, k, v, w_out, b_out)
    res = run_bass_kernel_spmd(
        nc, in_maps, core_ids=list(range(N_CORES)),
        trace=TRACE, **TRACE_KWARGS,
    )
    LAST_RESULT = res
    out = np.concatenate(
        [r["out"][None] for r in res.results], axis=0
    )  # [n_cores, hpc, S, 64]
    return out.reshape(B, H, S, 64)
